# revision 3
# baseline (speedup 1.0000x reference)
"""Self-contained Trainium2 Bass kernel for the 2-layer GCN + Student-t cluster
assignment encoder (N=100k nodes, E=1.6M edges, 128->128->64 feats, K=32).

Strategy (8 NeuronCores, SPMD, one shared program per launch):
  K0: per-core xw1T = W1^T @ xT for its node shard (feature-major I/O).
      Host concatenates the 8 shards into the full gather table xw1 [NPAD,128].
  K1: per-core GCN layer-1 aggregation over dst-sharded edges:
      dma_gather of xw1 rows (512B each), windowed one-hot selection matrix
      S = (colidx == slot_rel) * norm built on the vector engine, PE matmul
      gathered^T @ S accumulated in a [128,512] PSUM tile per 4-block
      superblock, then ReLU(+b1) -> hT shard.  Host concatenates full h.
  K2: same aggregation machinery over h, then z = W2^T @ agg + b2 and the
      Student-t soft assignment q, all on-chip.  Host transposes/concats.

All per-edge normalization (norm = dinv[src]*dinv[dst], a pure function of
edge_index) is host-side graph preprocessing baked into S.  The schedule
(tiles per group, PSUM window bases) is computed from the merged 8-core edge
distribution so the single SPMD program is valid for every core.
"""

import os
import time

import numpy as np

import concourse.bass as bass
import concourse.bacc as bacc
import concourse.tile as tile
from concourse import mybir
from concourse import bass_utils

F32 = mybir.dt.float32
I16 = mybir.dt.int16

NCORES = 8
P = 128          # partitions / tile lanes
WTILE = 64       # one-hot window width (slots) per edge tile


class Cfg:
    def __init__(self, N, IN=128, HID=128, OUT=64, K=32, V=2.0,
                 SBB=4, CHUNK=25088):
        self.N, self.IN, self.HID, self.OUT, self.K, self.V = N, IN, HID, OUT, K, V
        self.SBB = SBB                    # blocks per superblock
        self.SLOTS = SBB * P              # dst slots per superblock
        self.NSHARD = -(-N // (NCORES * P)) * P   # dst nodes per core (padded)
        self.NPAD = self.NSHARD * NCORES
        self.NBLK = self.NSHARD // P      # 128-node blocks per core
        self.NSB = -(-self.NBLK // SBB)   # superblocks per core
        self.CHUNK = min(CHUNK, self.NPAD)  # gather-table chunk rows (int16 idx)
        self.NCHUNK = -(-self.NPAD // self.CHUNK)


CFG = Cfg(N=100000)

# Filled by kernel() with per-launch HW exec times when BASSGNN_TRACE=1.
last_run_info = {}


# ---------------------------------------------------------------- host prep

def _preprocess(cfg, edge_index):
    """Shard + sort edges, build the shared tile schedule and per-core packed
    index/slot/norm arrays."""
    src = edge_index[0].astype(np.int64)
    dst = edge_index[1].astype(np.int64)
    loop = np.arange(cfg.N, dtype=np.int64)
    src2 = np.concatenate([src, loop])
    dst2 = np.concatenate([dst, loop])

    deg = np.bincount(dst2, minlength=cfg.N).astype(np.float32)
    dinv = (1.0 / np.sqrt(deg)).astype(np.float32)
    normv = dinv[src2] * dinv[dst2]

    core = dst2 // cfg.NSHARD
    local = dst2 - core * cfg.NSHARD
    sb = local // cfg.SLOTS
    slot = (local - sb * cfg.SLOTS).astype(np.int32)
    chunk = src2 // cfg.CHUNK
    lidx = (src2 - chunk * cfg.CHUNK).astype(np.int32)
    NG = cfg.NSB * cfg.NCHUNK
    g = sb * cfg.NCHUNK + chunk
    key = core * NG + g

    order = np.lexsort((slot, key))
    key_s = key[order]
    slot_s = slot[order]
    lidx_s = lidx[order]
    norm_s = normv[order]
    bounds = np.searchsorted(key_s, np.arange(NCORES * NG + 1))

    # ---- merged greedy tiling per group (shared windows across cores)
    group_tiles = []          # per g: list of (a, b) slot windows
    for gi in range(NG):
        sbi = gi // cfg.NCHUNK
        Wsb = min(cfg.SBB, cfg.NBLK - sbi * cfg.SBB) * P
        cnt = np.zeros((NCORES, Wsb + 1), np.int64)
        for c in range(NCORES):
            s0, s1 = bounds[c * NG + gi], bounds[c * NG + gi + 1]
            if s1 > s0:
                cnt[c, 1:] = np.bincount(slot_s[s0:s1], minlength=Wsb)
        Pm = cnt.cumsum(axis=1)
        tl = []
        a = 0
        while a < Wsb:
            hi = min(a + WTILE, Wsb)
            loads = (Pm[:, a + 1:hi + 1] - Pm[:, a:a + 1]).max(axis=0)
            nok = int(np.count_nonzero(loads <= P))
            if nok == 0:
                raise RuntimeError("slot with >128 edges on one core")
            b = a + nok
            if loads[nok - 1] > 0:
                tl.append((a, b))
            a = b
        group_tiles.append(tl)

    T_g = np.array([len(tl) for tl in group_tiles], np.int64)
    tile_off = np.zeros(NG + 1, np.int64)
    np.cumsum(T_g, out=tile_off[1:])
    TOT_TILES = int(tile_off[-1])

    slot_tiles = np.full((NCORES, TOT_TILES, P), -1.0, np.float32)
    norm_tiles = np.zeros((NCORES, TOT_TILES, P), np.float32)
    idx_tiles = np.zeros((NCORES, TOT_TILES, P), np.int16)
    bases_all = np.zeros(TOT_TILES, np.int64)

    for gi, tl in enumerate(group_tiles):
        if not tl:
            continue
        sbi = gi // cfg.NCHUNK
        Wsb = min(cfg.SBB, cfg.NBLK - sbi * cfg.SBB) * P
        toff = int(tile_off[gi])
        cuts = np.array([t[0] for t in tl] + [tl[-1][1]], np.int64)
        for t, (a, b) in enumerate(tl):
            bases_all[toff + t] = min(a, Wsb - WTILE)
        for c in range(NCORES):
            s0, s1 = bounds[c * NG + gi], bounds[c * NG + gi + 1]
            if s1 == s0:
                continue
            ss = slot_s[s0:s1]
            pos = np.searchsorted(ss, cuts) + s0
            for t in range(len(tl)):
                lo, hi = int(pos[t]), int(pos[t + 1])
                n = hi - lo
                if n == 0:
                    continue
                base = bases_all[toff + t]
                slot_tiles[c, toff + t, :n] = (slot_s[lo:hi] - base).astype(np.float32)
                norm_tiles[c, toff + t, :n] = norm_s[lo:hi]
                idx_tiles[c, toff + t, :n] = lidx_s[lo:hi].astype(np.int16)

    # device-layout arrays
    slotv = [np.ascontiguousarray(slot_tiles[c].T) for c in range(NCORES)]
    normd = [np.ascontiguousarray(norm_tiles[c].T) for c in range(NCORES)]
    idxw = [np.ascontiguousarray(
        np.tile(idx_tiles[c].reshape(-1, 16).T, (8, 1)))
        for c in range(NCORES)]

    # schedule: per sb, list of (chunk, Tg, toff) for nonempty groups
    sched = []
    for sbi in range(cfg.NSB):
        ent = []
        for ci in range(cfg.NCHUNK):
            gi = sbi * cfg.NCHUNK + ci
            if T_g[gi] > 0:
                ent.append((ci, int(T_g[gi]), int(tile_off[gi])))
        sched.append(ent)

    return dict(sched=sched, bases=bases_all, TOT_TILES=TOT_TILES,
                slotv=slotv, normv=normd, idxw=idxw)


# ------------------------------------------------------------- bass builders

def _build_k0(cfg):
    """xw1T = W1^T @ xT per block."""
    nc = bacc.Bacc("TRN2", target_bir_lowering=False, debug=False,
                   num_devices=NCORES)
    xT = nc.dram_tensor("xT", [P, cfg.NSHARD], F32, kind="ExternalInput").ap()
    w1 = nc.dram_tensor("w1", [cfg.IN, cfg.HID], F32, kind="ExternalInput").ap()
    out = nc.dram_tensor("xw1T", [cfg.HID, cfg.NSHARD], F32,
                         kind="ExternalOutput").ap()
    with tile.TileContext(nc) as tc:
        with (
            tc.tile_pool(name="sb", bufs=2) as pool,
            tc.tile_pool(name="cst", bufs=1) as cst,
            tc.tile_pool(name="ps", bufs=4, space="PSUM") as psp,
        ):
            xt = cst.tile([P, cfg.NSHARD], F32)
            nc.sync.dma_start(out=xt[:], in_=xT[:])
            w1t = cst.tile([cfg.IN, cfg.HID], F32)
            nc.sync.dma_start(out=w1t[:], in_=w1[:])
            for blk in range(cfg.NBLK):
                ps = psp.tile([cfg.HID, P], F32, tag="ps")
                nc.tensor.matmul(ps[:], lhsT=w1t[:],
                                 rhs=xt[:, blk * P:(blk + 1) * P],
                                 start=True, stop=True)
                ot = pool.tile([cfg.HID, P], F32, tag="ot")
                nc.scalar.copy(ot[:], ps[:])
                nc.sync.dma_start(out=out[:, blk * P:(blk + 1) * P], in_=ot[:])
    nc.compile()
    return nc


def _build_agg(cfg, prep, layer):
    """Aggregation kernel.  layer=1: out hT=relu(agg+b1).  layer=2: out zT,qT."""
    sched = prep["sched"]
    bases = prep["bases"]
    TOT = prep["TOT_TILES"]
    TOTIDX = TOT * P

    nc = bacc.Bacc("TRN2", target_bir_lowering=False, debug=False,
                   num_devices=NCORES)
    D = cfg.HID  # gather-table feature dim (128 for both layers)
    tbl = nc.dram_tensor("tbl", [cfg.NPAD, D], F32, kind="ExternalInput").ap()
    idxd = nc.dram_tensor("idxw", [P, TOTIDX // 16], I16,
                          kind="ExternalInput").ap()
    slotd = nc.dram_tensor("slotv", [P, TOT], F32, kind="ExternalInput").ap()
    normd = nc.dram_tensor("normv", [P, TOT], F32, kind="ExternalInput").ap()
    cold = nc.dram_tensor("colidx", [P, WTILE], F32, kind="ExternalInput").ap()
    if layer == 1:
        b1d = nc.dram_tensor("b1", [cfg.HID, 1], F32, kind="ExternalInput").ap()
        hTd = nc.dram_tensor("hT", [cfg.HID, cfg.NSHARD], F32,
                             kind="ExternalOutput").ap()
    else:
        w2d = nc.dram_tensor("w2", [cfg.HID, cfg.OUT], F32,
                             kind="ExternalInput").ap()
        b2d = nc.dram_tensor("b2", [cfg.OUT, 1], F32, kind="ExternalInput").ap()
        cm2d = nc.dram_tensor("cm2", [cfg.OUT, cfg.K], F32,
                              kind="ExternalInput").ap()
        cn2d = nc.dram_tensor("cn2", [cfg.K, 1], F32, kind="ExternalInput").ap()
        zTd = nc.dram_tensor("zT", [cfg.OUT, cfg.NSHARD], F32,
                             kind="ExternalOutput").ap()
        qTd = nc.dram_tensor("qT", [cfg.K, cfg.NSHARD], F32,
                             kind="ExternalOutput").ap()

    with tile.TileContext(nc) as tc:
        with (
            tc.tile_pool(name="cst", bufs=1) as cst,
            tc.tile_pool(name="gath", bufs=8) as gpool,
            tc.tile_pool(name="spool", bufs=4) as spool,
            tc.tile_pool(name="opool", bufs=4) as opool,
            tc.tile_pool(name="psagg", bufs=2, space="PSUM") as psagg,
            tc.tile_pool(name="pssm", bufs=4, space="PSUM") as pssm,
        ):
            idx_t = cst.tile([P, TOTIDX // 16], I16)
            nc.sync.dma_start(out=idx_t[:], in_=idxd[:])
            slot_t = cst.tile([P, TOT], F32)
            nc.sync.dma_start(out=slot_t[:], in_=slotd[:])
            norm_t = cst.tile([P, TOT], F32)
            nc.sync.dma_start(out=norm_t[:], in_=normd[:])
            col_t = cst.tile([P, WTILE], F32)
            nc.sync.dma_start(out=col_t[:], in_=cold[:])
            zrow = cst.tile([1, cfg.SLOTS], F32)
            nc.vector.memset(zrow[:], 0.0)
            zcol = cst.tile([1, P], F32)
            nc.vector.memset(zcol[:], 0.0)
            if layer == 1:
                b1_t = cst.tile([cfg.HID, 1], F32)
                nc.sync.dma_start(out=b1_t[:], in_=b1d[:])
            else:
                w2_t = cst.tile([cfg.HID, cfg.OUT], F32)
                nc.sync.dma_start(out=w2_t[:], in_=w2d[:])
                b2_t = cst.tile([cfg.OUT, 1], F32)
                nc.sync.dma_start(out=b2_t[:], in_=b2d[:])
                cm2_t = cst.tile([cfg.OUT, cfg.K], F32)
                nc.sync.dma_start(out=cm2_t[:], in_=cm2d[:])
                cn2_t = cst.tile([cfg.K, 1], F32)
                nc.sync.dma_start(out=cn2_t[:], in_=cn2d[:])
                ones_h1 = cst.tile([cfg.OUT, 1], F32)
                nc.vector.memset(ones_h1[:], 1.0)
                ones_1k = cst.tile([1, cfg.K], F32)
                nc.vector.memset(ones_1k[:], 1.0)
                ones_k1 = cst.tile([cfg.K, 1], F32)
                nc.vector.memset(ones_k1[:], 1.0)

            for sbi in range(cfg.NSB):
                nb = min(cfg.SBB, cfg.NBLK - sbi * cfg.SBB)
                W = nb * P
                ps = psagg.tile([P, cfg.SLOTS], F32, tag="agg")
                # zero-init + set has_written over the full width
                nc.tensor.matmul(ps[:, :W], lhsT=zcol[:1, :],
                                 rhs=zrow[:1, :W], start=True, stop=False,
                                 skip_group_check=True)
                ents = sched[sbi]
                ntiles_sb = sum(e[1] for e in ents)
                k = 0
                for (ci, Tg, toff) in ents:
                    gt = gpool.tile([P, Tg, D], F32, tag="g")
                    c0 = ci * cfg.CHUNK
                    c1 = min(c0 + cfg.CHUNK, cfg.NPAD)
                    nc.gpsimd.dma_gather(
                        out_ap=gt[:],
                        in_ap=tbl[c0:c1, :],
                        idxs_ap=idx_t[:, toff * 8:(toff + Tg) * 8],
                        num_idxs=Tg * P,
                        num_idxs_reg=Tg * P,
                        elem_size=D,
                        single_packet=False,
                    )
                    for j in range(Tg):
                        tcol = toff + j
                        base = int(bases[tcol])
                        S = spool.tile([P, WTILE], F32, tag="S")
                        nc.vector.tensor_scalar(
                            out=S[:], in0=col_t[:],
                            scalar1=slot_t[:, tcol:tcol + 1],
                            scalar2=norm_t[:, tcol:tcol + 1],
                            op0=mybir.AluOpType.is_equal,
                            op1=mybir.AluOpType.mult,
                        )
                        k += 1
                        nc.tensor.matmul(
                            ps[:, base:base + WTILE], lhsT=gt[:, j, :],
                            rhs=S[:], start=False, stop=(k == ntiles_sb),
                            skip_group_check=True,
                        )
                for b in range(nb):
                    blk = sbi * cfg.SBB + b
                    pv = ps[:, b * P:(b + 1) * P]
                    if layer == 1:
                        ht = opool.tile([cfg.HID, P], F32, tag="hT")
                        nc.scalar.activation(
                            ht[:], pv, mybir.ActivationFunctionType.Relu,
                            bias=b1_t[:, 0:1], scale=1.0)
                        nc.sync.dma_start(
                            out=hTd[:, blk * P:(blk + 1) * P], in_=ht[:])
                    else:
                        ag = opool.tile([cfg.HID, P], F32, tag="ag")
                        nc.scalar.copy(ag[:], pv)
                        zps = pssm.tile([cfg.OUT, P], F32, tag="sm")
                        nc.tensor.matmul(zps[:], lhsT=w2_t[:], rhs=ag[:],
                                         start=True, stop=True)
                        zt = opool.tile([cfg.OUT, P], F32, tag="zT")
                        nc.scalar.activation(
                            zt[:], zps[:],
                            mybir.ActivationFunctionType.Identity,
                            bias=b2_t[:, 0:1], scale=1.0)
                        nc.sync.dma_start(
                            out=zTd[:, blk * P:(blk + 1) * P], in_=zt[:])
                        # Student-t q
                        zsq = opool.tile([cfg.OUT, P], F32, tag="zsq")
                        nc.scalar.activation(
                            zsq[:], zt[:], mybir.ActivationFunctionType.Square)
                        nps = pssm.tile([1, P], F32, tag="sm")
                        nc.tensor.matmul(nps[:], lhsT=ones_h1[:, 0:1],
                                         rhs=zsq[:], start=True, stop=True)
                        n1 = opool.tile([1, P], F32, tag="n1")
                        nc.vector.tensor_copy(n1[:], nps[:])
                        qps = pssm.tile([cfg.K, P], F32, tag="sm")
                        nc.tensor.matmul(qps[:], lhsT=ones_1k[:1, :], rhs=n1[:1, :],
                                         start=True, stop=False)
                        nc.tensor.matmul(qps[:], lhsT=cm2_t[:], rhs=zt[:],
                                         start=False, stop=True)
                        squ = opool.tile([cfg.K, P], F32, tag="squ")
                        nc.vector.tensor_scalar(
                            out=squ[:], in0=qps[:],
                            scalar1=cn2_t[:, 0:1], scalar2=0.0,
                            op0=mybir.AluOpType.add,
                            op1=mybir.AluOpType.max)
                        ut = opool.tile([cfg.K, P], F32, tag="ut")
                        nc.scalar.activation(
                            ut[:], squ[:],
                            mybir.ActivationFunctionType.Identity,
                            bias=1.0, scale=1.0 / cfg.V)
                        tt = opool.tile([cfg.K, P], F32, tag="tt")
                        nc.vector.reciprocal(tt[:], ut[:])
                        st = opool.tile([cfg.K, P], F32, tag="st")
                        nc.scalar.activation(
                            st[:], tt[:], mybir.ActivationFunctionType.Sqrt)
                        qu = opool.tile([cfg.K, P], F32, tag="qu")
                        nc.vector.tensor_mul(qu[:], tt[:], st[:])
                        sps = pssm.tile([1, P], F32, tag="sm")
                        nc.tensor.matmul(sps[:], lhsT=ones_k1[:, 0:1],
                                         rhs=qu[:], start=True, stop=True)
                        s1 = opool.tile([1, P], F32, tag="s1")
                        nc.vector.tensor_copy(s1[:], sps[:])
                        r1 = opool.tile([1, P], F32, tag="r1")
                        nc.vector.reciprocal(r1[:], s1[:])
                        bps = pssm.tile([cfg.K, P], F32, tag="sm")
                        nc.tensor.matmul(bps[:], lhsT=ones_1k[:1, :],
                                         rhs=r1[:1, :], start=True, stop=True)
                        qf = opool.tile([cfg.K, P], F32, tag="qf")
                        nc.vector.tensor_mul(qf[:], qu[:], bps[:])
                        nc.sync.dma_start(
                            out=qTd[:, blk * P:(blk + 1) * P], in_=qf[:])
    nc.compile()
    return nc


# ------------------------------------------------------------------- driver

def _run(nc, in_maps, trace, label):
    kw = {}
    if trace:
        tdir = os.path.join("/tmp", f"bassgnn_trace_{label}")
        os.makedirs(tdir, exist_ok=True)
        kw["tmpdir"] = tdir
    res = bass_utils.run_bass_kernel_spmd(
        nc, in_maps, core_ids=list(range(NCORES)), trace=trace, **kw)
    if trace:
        last_run_info[label] = res.exec_time_ns
        last_run_info[label + "_dir"] = kw.get("tmpdir")
    return res.results


def _kernel_impl(cfg, x, edge_index, W1, b1, W2, b2, cluster):
    trace = os.environ.get("BASSGNN_TRACE", "0") == "1"
    last_run_info.clear()

    x = np.asarray(x, np.float32)
    W1 = np.asarray(W1, np.float32)
    b1 = np.asarray(b1, np.float32)
    W2 = np.asarray(W2, np.float32)
    b2 = np.asarray(b2, np.float32)
    cluster = np.asarray(cluster, np.float32)
    ei = np.asarray(edge_index)

    t0 = time.time()
    prep = _preprocess(cfg, ei)
    t1 = time.time()

    nc0 = _build_k0(cfg)
    nc1 = _build_agg(cfg, prep, layer=1)
    nc2 = _build_agg(cfg, prep, layer=2)
    t2 = time.time()

    xpad = np.zeros((cfg.NPAD, cfg.IN), np.float32)
    xpad[:cfg.N] = x
    colidx = np.broadcast_to(
        np.arange(WTILE, dtype=np.float32), (P, WTILE)).copy()

    # K0
    in0 = [{"xT": np.ascontiguousarray(
                xpad[c * cfg.NSHARD:(c + 1) * cfg.NSHARD].T),
            "w1": W1} for c in range(NCORES)]
    r0 = _run(nc0, in0, trace, "k0")
    xw1 = np.concatenate([r0[c]["xw1T"].T for c in range(NCORES)], axis=0)

    # K1
    in1 = [{"tbl": xw1, "idxw": prep["idxw"][c], "slotv": prep["slotv"][c],
            "normv": prep["normv"][c], "colidx": colidx,
            "b1": b1.reshape(-1, 1)} for c in range(NCORES)]
    r1 = _run(nc1, in1, trace, "k1")
    h = np.concatenate([r1[c]["hT"].T for c in range(NCORES)], axis=0)

    # K2
    cm2 = np.ascontiguousarray((-2.0 * cluster.T))
    cn2 = (cluster * cluster).sum(axis=1).reshape(-1, 1).astype(np.float32)
    in2 = [{"tbl": h, "idxw": prep["idxw"][c], "slotv": prep["slotv"][c],
            "normv": prep["normv"][c], "colidx": colidx,
            "w2": W2, "b2": b2.reshape(-1, 1), "cm2": cm2, "cn2": cn2}
           for c in range(NCORES)]
    r2 = _run(nc2, in2, trace, "k2")
    z = np.concatenate([r2[c]["zT"].T for c in range(NCORES)], axis=0)[:cfg.N]
    q = np.concatenate([r2[c]["qT"].T for c in range(NCORES)], axis=0)[:cfg.N]

    t3 = time.time()
    last_run_info["prep_s"] = t1 - t0
    last_run_info["build_s"] = t2 - t1
    last_run_info["run_s"] = t3 - t2
    return np.ascontiguousarray(z), np.ascontiguousarray(q)


def kernel(x, edge_index, W1, b1, W2, b2, cluster):
    return _kernel_impl(CFG, x, edge_index, W1, b1, W2, b2, cluster)


# revision 5
# speedup vs baseline: 1.2612x; 1.2612x over previous
"""Self-contained Trainium2 Bass kernel for the 2-layer GCN + Student-t cluster
assignment encoder (N=100k nodes, E=1.6M edges, 128->128->64 feats, K=32).

Strategy (8 NeuronCores, SPMD, one shared program per launch):
  K0: per-core xw1T = W1^T @ xT for its node shard (feature-major I/O).
      Host concatenates the 8 shards into the full gather table xw1 [NPAD,128].
  K1: per-core GCN layer-1 aggregation over dst-sharded edges:
      dma_gather of xw1 rows (512B each), windowed one-hot selection matrix
      S = (colidx == slot_rel) * norm built on the vector engine, PE matmul
      gathered^T @ S accumulated in a [128,512] PSUM tile per 4-block
      superblock, then ReLU(+b1) -> hT shard.  Host concatenates full h.
  K2: same aggregation machinery over h, then z = W2^T @ agg + b2 and the
      Student-t soft assignment q, all on-chip.  Host transposes/concats.

All per-edge normalization (norm = dinv[src]*dinv[dst], a pure function of
edge_index) is host-side graph preprocessing baked into S.  The schedule
(tiles per group, PSUM window bases) is computed from the merged 8-core edge
distribution so the single SPMD program is valid for every core.
"""

import os
import time

import numpy as np

import concourse.bass as bass
import concourse.bacc as bacc
import concourse.tile as tile
from concourse import mybir
from concourse import bass_utils

F32 = mybir.dt.float32
BF16 = mybir.dt.bfloat16
I16 = mybir.dt.int16

import ml_dtypes
BF16NP = ml_dtypes.bfloat16

NCORES = 8
P = 128          # partitions / tile lanes
WTILE = 64       # one-hot window width (slots) per edge tile


class Cfg:
    def __init__(self, N, IN=128, HID=128, OUT=64, K=32, V=2.0,
                 SBB=4, CHUNK=25088):
        self.N, self.IN, self.HID, self.OUT, self.K, self.V = N, IN, HID, OUT, K, V
        self.SBB = SBB                    # blocks per superblock
        self.SLOTS = SBB * P              # dst slots per superblock
        self.NSHARD = -(-N // (NCORES * P)) * P   # dst nodes per core (padded)
        self.NPAD = self.NSHARD * NCORES
        self.NBLK = self.NSHARD // P      # 128-node blocks per core
        self.NSB = -(-self.NBLK // SBB)   # superblocks per core
        self.CHUNK = min(CHUNK, self.NPAD)  # gather-table chunk rows (int16 idx)
        self.NCHUNK = -(-self.NPAD // self.CHUNK)


CFG = Cfg(N=100000)

# Filled by kernel() with per-launch HW exec times when BASSGNN_TRACE=1.
last_run_info = {}


# ---------------------------------------------------------------- host prep

def _preprocess(cfg, edge_index):
    """Shard + sort edges, build the shared tile schedule and per-core packed
    index/slot/norm arrays."""
    src = edge_index[0].astype(np.int64)
    dst = edge_index[1].astype(np.int64)
    loop = np.arange(cfg.N, dtype=np.int64)
    src2 = np.concatenate([src, loop])
    dst2 = np.concatenate([dst, loop])

    deg = np.bincount(dst2, minlength=cfg.N).astype(np.float32)
    dinv = (1.0 / np.sqrt(deg)).astype(np.float32)
    normv = dinv[src2] * dinv[dst2]

    core = dst2 // cfg.NSHARD
    local = dst2 - core * cfg.NSHARD
    sb = local // cfg.SLOTS
    slot = (local - sb * cfg.SLOTS).astype(np.int32)
    chunk = src2 // cfg.CHUNK
    lidx = (src2 - chunk * cfg.CHUNK).astype(np.int32)
    NG = cfg.NSB * cfg.NCHUNK
    g = sb * cfg.NCHUNK + chunk
    key = core * NG + g

    order = np.lexsort((slot, key))
    key_s = key[order]
    slot_s = slot[order]
    lidx_s = lidx[order]
    norm_s = normv[order]
    bounds = np.searchsorted(key_s, np.arange(NCORES * NG + 1))

    # ---- merged greedy tiling per group (shared windows across cores)
    group_tiles = []          # per g: list of (a, b) slot windows
    for gi in range(NG):
        sbi = gi // cfg.NCHUNK
        Wsb = min(cfg.SBB, cfg.NBLK - sbi * cfg.SBB) * P
        cnt = np.zeros((NCORES, Wsb + 1), np.int64)
        for c in range(NCORES):
            s0, s1 = bounds[c * NG + gi], bounds[c * NG + gi + 1]
            if s1 > s0:
                cnt[c, 1:] = np.bincount(slot_s[s0:s1], minlength=Wsb)
        Pm = cnt.cumsum(axis=1)
        tl = []
        a = 0
        while a < Wsb:
            hi = min(a + WTILE, Wsb)
            loads = (Pm[:, a + 1:hi + 1] - Pm[:, a:a + 1]).max(axis=0)
            nok = int(np.count_nonzero(loads <= P))
            if nok == 0:
                raise RuntimeError("slot with >128 edges on one core")
            b = a + nok
            if loads[nok - 1] > 0:
                tl.append((a, b))
            a = b
        group_tiles.append(tl)

    T_g = np.array([len(tl) for tl in group_tiles], np.int64)
    tile_off = np.zeros(NG + 1, np.int64)
    np.cumsum(T_g, out=tile_off[1:])
    TOT_TILES = int(tile_off[-1])

    slot_tiles = np.full((NCORES, TOT_TILES, P), -1.0, np.float32)
    norm_tiles = np.zeros((NCORES, TOT_TILES, P), np.float32)
    idx_tiles = np.zeros((NCORES, TOT_TILES, P), np.int16)
    bases_all = np.zeros(TOT_TILES, np.int64)

    for gi, tl in enumerate(group_tiles):
        if not tl:
            continue
        sbi = gi // cfg.NCHUNK
        Wsb = min(cfg.SBB, cfg.NBLK - sbi * cfg.SBB) * P
        toff = int(tile_off[gi])
        cuts = np.array([t[0] for t in tl] + [tl[-1][1]], np.int64)
        for t, (a, b) in enumerate(tl):
            bases_all[toff + t] = min(a, Wsb - WTILE)
        for c in range(NCORES):
            s0, s1 = bounds[c * NG + gi], bounds[c * NG + gi + 1]
            if s1 == s0:
                continue
            ss = slot_s[s0:s1]
            pos = np.searchsorted(ss, cuts) + s0
            for t in range(len(tl)):
                lo, hi = int(pos[t]), int(pos[t + 1])
                n = hi - lo
                if n == 0:
                    continue
                base = bases_all[toff + t]
                slot_tiles[c, toff + t, :n] = (slot_s[lo:hi] - base).astype(np.float32)
                norm_tiles[c, toff + t, :n] = norm_s[lo:hi]
                idx_tiles[c, toff + t, :n] = lidx_s[lo:hi].astype(np.int16)

    # device-layout arrays ([P, TOT, 1] bf16)
    slotv = [np.ascontiguousarray(slot_tiles[c].T)[:, :, None].astype(BF16NP)
             for c in range(NCORES)]
    normd = [np.ascontiguousarray(norm_tiles[c].T)[:, :, None].astype(BF16NP)
             for c in range(NCORES)]
    idxw = [np.ascontiguousarray(
        np.tile(idx_tiles[c].reshape(-1, 16).T, (8, 1)))
        for c in range(NCORES)]

    # schedule: per sb, list of (chunk, Tg, toff) for nonempty groups
    sched = []
    for sbi in range(cfg.NSB):
        ent = []
        for ci in range(cfg.NCHUNK):
            gi = sbi * cfg.NCHUNK + ci
            if T_g[gi] > 0:
                ent.append((ci, int(T_g[gi]), int(tile_off[gi])))
        sched.append(ent)

    return dict(sched=sched, bases=bases_all, TOT_TILES=TOT_TILES,
                slotv=slotv, normv=normd, idxw=idxw)


# ------------------------------------------------------------- bass builders

def _build_k0(cfg):
    """xw1T = W1^T @ xT per block."""
    nc = bacc.Bacc("TRN2", target_bir_lowering=False, debug=False,
                   num_devices=NCORES)
    xT = nc.dram_tensor("xT", [P, cfg.NSHARD], F32, kind="ExternalInput").ap()
    w1 = nc.dram_tensor("w1", [cfg.IN, cfg.HID], F32, kind="ExternalInput").ap()
    out = nc.dram_tensor("xw1T", [cfg.HID, cfg.NSHARD], BF16,
                         kind="ExternalOutput").ap()
    with tile.TileContext(nc) as tc:
        with (
            tc.tile_pool(name="sb", bufs=2) as pool,
            tc.tile_pool(name="cst", bufs=1) as cst,
            tc.tile_pool(name="ps", bufs=4, space="PSUM") as psp,
        ):
            xt = cst.tile([P, cfg.NSHARD], F32)
            nc.sync.dma_start(out=xt[:], in_=xT[:])
            w1t = cst.tile([cfg.IN, cfg.HID], F32)
            nc.sync.dma_start(out=w1t[:], in_=w1[:])
            for blk in range(cfg.NBLK):
                ps = psp.tile([cfg.HID, P], F32, tag="ps")
                nc.tensor.matmul(ps[:], lhsT=w1t[:],
                                 rhs=xt[:, blk * P:(blk + 1) * P],
                                 start=True, stop=True)
                ot = pool.tile([cfg.HID, P], BF16, tag="ot")
                nc.scalar.copy(ot[:], ps[:])
                nc.sync.dma_start(out=out[:, blk * P:(blk + 1) * P], in_=ot[:])
    nc.compile()
    return nc


def _build_agg(cfg, prep, layer):
    """Aggregation kernel.  layer=1: out hT=relu(agg+b1).  layer=2: out zT,qT."""
    sched = prep["sched"]
    bases = prep["bases"]
    TOT = prep["TOT_TILES"]
    TOTIDX = TOT * P

    nc = bacc.Bacc("TRN2", target_bir_lowering=False, debug=False,
                   num_devices=NCORES)
    D = cfg.HID  # gather-table feature dim (128 for both layers)
    tbl = nc.dram_tensor("tbl", [cfg.NPAD, D], BF16, kind="ExternalInput").ap()
    idxd = nc.dram_tensor("idxw", [P, TOTIDX // 16], I16,
                          kind="ExternalInput").ap()
    slotd = nc.dram_tensor("slotv", [P, TOT, 1], BF16, kind="ExternalInput").ap()
    normd = nc.dram_tensor("normv", [P, TOT, 1], BF16, kind="ExternalInput").ap()
    cold = nc.dram_tensor("colidx", [P, 1, WTILE], BF16, kind="ExternalInput").ap()
    if layer == 1:
        b1d = nc.dram_tensor("b1", [cfg.HID, 1], F32, kind="ExternalInput").ap()
        hTd = nc.dram_tensor("hT", [cfg.HID, cfg.NSHARD], BF16,
                             kind="ExternalOutput").ap()
    else:
        w2d = nc.dram_tensor("w2", [cfg.HID, cfg.OUT], F32,
                             kind="ExternalInput").ap()
        b2d = nc.dram_tensor("b2", [cfg.OUT, 1], F32, kind="ExternalInput").ap()
        cm2d = nc.dram_tensor("cm2", [cfg.OUT, cfg.K], F32,
                              kind="ExternalInput").ap()
        cn2d = nc.dram_tensor("cn2", [cfg.K, 1], F32, kind="ExternalInput").ap()
        zTd = nc.dram_tensor("zT", [cfg.OUT, cfg.NSHARD], F32,
                             kind="ExternalOutput").ap()
        qTd = nc.dram_tensor("qT", [cfg.K, cfg.NSHARD], F32,
                             kind="ExternalOutput").ap()

    with tile.TileContext(nc) as tc:
        with (
            tc.tile_pool(name="cst", bufs=1) as cst,
            tc.tile_pool(name="gath", bufs=8) as gpool,
            tc.tile_pool(name="spool", bufs=4) as spool,
            tc.tile_pool(name="opool", bufs=4) as opool,
            tc.tile_pool(name="psagg", bufs=2, space="PSUM") as psagg,
            tc.tile_pool(name="pssm", bufs=4, space="PSUM") as pssm,
        ):
            idx_t = cst.tile([P, TOTIDX // 16], I16)
            nc.sync.dma_start(out=idx_t[:], in_=idxd[:])
            slot_t = cst.tile([P, TOT, 1], BF16)
            nc.sync.dma_start(out=slot_t[:], in_=slotd[:])
            norm_t = cst.tile([P, TOT, 1], BF16)
            nc.sync.dma_start(out=norm_t[:], in_=normd[:])
            col_t = cst.tile([P, 1, WTILE], BF16)
            nc.sync.dma_start(out=col_t[:], in_=cold[:])
            zrow = cst.tile([1, cfg.SLOTS], F32)
            nc.vector.memset(zrow[:], 0.0)
            zcol = cst.tile([1, P], F32)
            nc.vector.memset(zcol[:], 0.0)
            if layer == 1:
                b1_t = cst.tile([cfg.HID, 1], F32)
                nc.sync.dma_start(out=b1_t[:], in_=b1d[:])
            else:
                w2_t = cst.tile([cfg.HID, cfg.OUT], F32)
                nc.sync.dma_start(out=w2_t[:], in_=w2d[:])
                b2_t = cst.tile([cfg.OUT, 1], F32)
                nc.sync.dma_start(out=b2_t[:], in_=b2d[:])
                cm2_t = cst.tile([cfg.OUT, cfg.K], F32)
                nc.sync.dma_start(out=cm2_t[:], in_=cm2d[:])
                cn2_t = cst.tile([cfg.K, 1], F32)
                nc.sync.dma_start(out=cn2_t[:], in_=cn2d[:])
                ones_h1 = cst.tile([cfg.OUT, 1], F32)
                nc.vector.memset(ones_h1[:], 1.0)
                ones_1k = cst.tile([1, cfg.K], F32)
                nc.vector.memset(ones_1k[:], 1.0)
                ones_k1 = cst.tile([cfg.K, 1], F32)
                nc.vector.memset(ones_k1[:], 1.0)

            for sbi in range(cfg.NSB):
                nb = min(cfg.SBB, cfg.NBLK - sbi * cfg.SBB)
                W = nb * P
                ps = psagg.tile([P, cfg.SLOTS], F32, tag="agg")
                # zero-init + set has_written over the full width
                nc.tensor.matmul(ps[:, :W], lhsT=zcol[:1, :],
                                 rhs=zrow[:1, :W], start=True, stop=False,
                                 skip_group_check=True)
                ents = sched[sbi]
                ntiles_sb = sum(e[1] for e in ents)
                k = 0
                for (ci, Tg, toff) in ents:
                    gt = gpool.tile([P, Tg, D], BF16, tag="g")
                    c0 = ci * cfg.CHUNK
                    c1 = min(c0 + cfg.CHUNK, cfg.NPAD)
                    nc.gpsimd.dma_gather(
                        out_ap=gt[:],
                        in_ap=tbl[c0:c1, :],
                        idxs_ap=idx_t[:, toff * 8:(toff + Tg) * 8],
                        num_idxs=Tg * P,
                        num_idxs_reg=Tg * P,
                        elem_size=D,
                        single_packet=False,
                    )
                    bshape = [P, Tg, WTILE]
                    seq = spool.tile(bshape, BF16, tag="Seq")
                    nc.vector.tensor_tensor(
                        out=seq[:], in0=col_t[:].to_broadcast(bshape),
                        in1=slot_t[:, toff:toff + Tg, :].to_broadcast(bshape),
                        op=mybir.AluOpType.is_equal)
                    sb_ = spool.tile(bshape, BF16, tag="S")
                    nc.vector.tensor_tensor(
                        out=sb_[:], in0=seq[:],
                        in1=norm_t[:, toff:toff + Tg, :].to_broadcast(bshape),
                        op=mybir.AluOpType.mult)
                    for j in range(Tg):
                        base = int(bases[toff + j])
                        k += 1
                        nc.tensor.matmul(
                            ps[:, base:base + WTILE], lhsT=gt[:, j, :],
                            rhs=sb_[:, j, :], start=False, stop=(k == ntiles_sb),
                            skip_group_check=True,
                        )
                for b in range(nb):
                    blk = sbi * cfg.SBB + b
                    pv = ps[:, b * P:(b + 1) * P]
                    if layer == 1:
                        ht = opool.tile([cfg.HID, P], BF16, tag="hT")
                        nc.scalar.activation(
                            ht[:], pv, mybir.ActivationFunctionType.Relu,
                            bias=b1_t[:, 0:1], scale=1.0)
                        nc.sync.dma_start(
                            out=hTd[:, blk * P:(blk + 1) * P], in_=ht[:])
                    else:
                        ag = opool.tile([cfg.HID, P], F32, tag="ag")
                        nc.scalar.copy(ag[:], pv)
                        zps = pssm.tile([cfg.OUT, P], F32, tag="sm")
                        nc.tensor.matmul(zps[:], lhsT=w2_t[:], rhs=ag[:],
                                         start=True, stop=True)
                        zt = opool.tile([cfg.OUT, P], F32, tag="zT")
                        nc.scalar.activation(
                            zt[:], zps[:],
                            mybir.ActivationFunctionType.Identity,
                            bias=b2_t[:, 0:1], scale=1.0)
                        nc.sync.dma_start(
                            out=zTd[:, blk * P:(blk + 1) * P], in_=zt[:])
                        # Student-t q
                        zsq = opool.tile([cfg.OUT, P], F32, tag="zsq")
                        nc.scalar.activation(
                            zsq[:], zt[:], mybir.ActivationFunctionType.Square)
                        nps = pssm.tile([1, P], F32, tag="sm")
                        nc.tensor.matmul(nps[:], lhsT=ones_h1[:, 0:1],
                                         rhs=zsq[:], start=True, stop=True)
                        n1 = opool.tile([1, P], F32, tag="n1")
                        nc.vector.tensor_copy(n1[:], nps[:])
                        qps = pssm.tile([cfg.K, P], F32, tag="sm")
                        nc.tensor.matmul(qps[:], lhsT=ones_1k[:1, :], rhs=n1[:1, :],
                                         start=True, stop=False)
                        nc.tensor.matmul(qps[:], lhsT=cm2_t[:], rhs=zt[:],
                                         start=False, stop=True)
                        squ = opool.tile([cfg.K, P], F32, tag="squ")
                        nc.vector.tensor_scalar(
                            out=squ[:], in0=qps[:],
                            scalar1=cn2_t[:, 0:1], scalar2=0.0,
                            op0=mybir.AluOpType.add,
                            op1=mybir.AluOpType.max)
                        ut = opool.tile([cfg.K, P], F32, tag="ut")
                        nc.scalar.activation(
                            ut[:], squ[:],
                            mybir.ActivationFunctionType.Identity,
                            bias=1.0, scale=1.0 / cfg.V)
                        tt = opool.tile([cfg.K, P], F32, tag="tt")
                        nc.vector.reciprocal(tt[:], ut[:])
                        st = opool.tile([cfg.K, P], F32, tag="st")
                        nc.scalar.activation(
                            st[:], tt[:], mybir.ActivationFunctionType.Sqrt)
                        qu = opool.tile([cfg.K, P], F32, tag="qu")
                        nc.vector.tensor_mul(qu[:], tt[:], st[:])
                        sps = pssm.tile([1, P], F32, tag="sm")
                        nc.tensor.matmul(sps[:], lhsT=ones_k1[:, 0:1],
                                         rhs=qu[:], start=True, stop=True)
                        s1 = opool.tile([1, P], F32, tag="s1")
                        nc.vector.tensor_copy(s1[:], sps[:])
                        r1 = opool.tile([1, P], F32, tag="r1")
                        nc.vector.reciprocal(r1[:], s1[:])
                        bps = pssm.tile([cfg.K, P], F32, tag="sm")
                        nc.tensor.matmul(bps[:], lhsT=ones_1k[:1, :],
                                         rhs=r1[:1, :], start=True, stop=True)
                        qf = opool.tile([cfg.K, P], F32, tag="qf")
                        nc.vector.tensor_mul(qf[:], qu[:], bps[:])
                        nc.sync.dma_start(
                            out=qTd[:, blk * P:(blk + 1) * P], in_=qf[:])
    nc.compile()
    return nc


# ------------------------------------------------------------------- driver

def _run(nc, in_maps, trace, label):
    kw = {}
    if trace:
        import shutil
        tdir = os.path.join("/tmp", f"bassgnn_trace_{label}")
        shutil.rmtree(tdir, ignore_errors=True)
        os.makedirs(tdir, exist_ok=True)
        kw["tmpdir"] = tdir
    res = bass_utils.run_bass_kernel_spmd(
        nc, in_maps, core_ids=list(range(NCORES)), trace=trace, **kw)
    if trace:
        last_run_info[label] = res.exec_time_ns
        last_run_info[label + "_dir"] = kw.get("tmpdir")
    return res.results


def _kernel_impl(cfg, x, edge_index, W1, b1, W2, b2, cluster):
    trace = os.environ.get("BASSGNN_TRACE", "0") == "1"
    last_run_info.clear()

    x = np.asarray(x, np.float32)
    W1 = np.asarray(W1, np.float32)
    b1 = np.asarray(b1, np.float32)
    W2 = np.asarray(W2, np.float32)
    b2 = np.asarray(b2, np.float32)
    cluster = np.asarray(cluster, np.float32)
    ei = np.asarray(edge_index)

    t0 = time.time()
    prep = _preprocess(cfg, ei)
    t1 = time.time()

    nc0 = _build_k0(cfg)
    nc1 = _build_agg(cfg, prep, layer=1)
    nc2 = _build_agg(cfg, prep, layer=2)
    t2 = time.time()

    xpad = np.zeros((cfg.NPAD, cfg.IN), np.float32)
    xpad[:cfg.N] = x
    colidx = np.broadcast_to(
        np.arange(WTILE, dtype=np.float32), (P, WTILE)).astype(BF16NP)[:, None, :].copy()

    # K0
    in0 = [{"xT": np.ascontiguousarray(
                xpad[c * cfg.NSHARD:(c + 1) * cfg.NSHARD].T),
            "w1": W1} for c in range(NCORES)]
    r0 = _run(nc0, in0, trace, "k0")
    xw1 = np.concatenate([r0[c]["xw1T"].T for c in range(NCORES)], axis=0)

    # K1
    in1 = [{"tbl": xw1, "idxw": prep["idxw"][c], "slotv": prep["slotv"][c],
            "normv": prep["normv"][c], "colidx": colidx,
            "b1": b1.reshape(-1, 1)} for c in range(NCORES)]
    r1 = _run(nc1, in1, trace, "k1")
    h = np.concatenate([r1[c]["hT"].T for c in range(NCORES)], axis=0)

    # K2
    cm2 = np.ascontiguousarray((-2.0 * cluster.T))
    cn2 = (cluster * cluster).sum(axis=1).reshape(-1, 1).astype(np.float32)
    in2 = [{"tbl": h, "idxw": prep["idxw"][c], "slotv": prep["slotv"][c],
            "normv": prep["normv"][c], "colidx": colidx,
            "w2": W2, "b2": b2.reshape(-1, 1), "cm2": cm2, "cn2": cn2}
           for c in range(NCORES)]
    r2 = _run(nc2, in2, trace, "k2")
    z = np.concatenate([r2[c]["zT"].T for c in range(NCORES)], axis=0)[:cfg.N]
    q = np.concatenate([r2[c]["qT"].T for c in range(NCORES)], axis=0)[:cfg.N]

    t3 = time.time()
    last_run_info["prep_s"] = t1 - t0
    last_run_info["build_s"] = t2 - t1
    last_run_info["run_s"] = t3 - t2
    return np.ascontiguousarray(z), np.ascontiguousarray(q)


def kernel(x, edge_index, W1, b1, W2, b2, cluster):
    return _kernel_impl(CFG, x, edge_index, W1, b1, W2, b2, cluster)
